# revision 21
# baseline (speedup 1.0000x reference)
"""Trainium2 Bass kernel for nn_BarcodeSLayerEncoder.

Design (8 NeuronCores, pure data-parallel over batch):
  - Each core gets B/8 = 32 batches of both barcode tensors.
  - SLayer logits via TensorE matmuls (K=120, 512 cols per batch):
      logit[16g+e, p] = 2*s_x*c_x[e]*x - s_x*x^2 + 2*s_y*c_y[e]*y - s_y*y^2
                        - 100*maskflag(g,p)          (g = 4 h0-chunks + 4 h1-chunks)
    with the per-center constant -s*||c||^2 folded into the ScalarE Exp bias.
    Supertiles are processed in blocks of up to 4 batches so one wide Exp
    activation covers [128, 2048] (amortizes the ACT access latency).
  - Point sums: 3 bf16 TensorTensor halvings on DVE (4x perf mode) then one
    grouped f32 tensor_reduce -> s_all f32 (bf16 rounding stays ~1e-3 overall).
  - Features x2 [32,32] f32 -> AllGather -> every core redundantly computes
    the tiny head with exact global BN stats (bn_stats/bn_aggr for mean+var).
    Head matmuls use float32r (full precision, 1 cycle/row at >=256 cols).
  - Act tables: a dummy Exp at t=0 preloads the exp table under the DMA
    prologue; a dummy Sqrt right after the last Exp switches to the sqrt
    table under the collective. Dummy matmuls keep the PE p-state at full
    clock through the collective so the head matmuls run at 2.4 GHz.
"""

import sys

sys.path.insert(0, "/opt/trn_rl_repo")

import numpy as np

N_CORES = 8
B, P, E, D = 256, 2048, 16, 2
BL = B // N_CORES  # 32 local batches per core
HID, OUT = 128, 128
BN_EPS = 1e-5
CH = 512  # point chunk size (one PSUM bank of f32)
NCH = P // CH  # 4 chunks per homology
NSUP = BL  # one batch per supertile
MASK_W = -100.0
BLOCK_SIZES = [1, 2, 3, 4, 4, 4, 4, 4, 3, 1, 1, 1]  # supertiles per exp block
NDUMMY = 92  # PE keep-warm matmuls spanning the collective window

# cpack (f32) column layout: ebias, g1, b1, g2, b2
EB_C = 0
G1_C, B1_C, G2_C, B2_C = 1, 2, 3, 4
CPACK_W = 5
# mpack (f32r) column layout: matmul weights for the head
WR_C = 0  # [0:128] WR = rmat @ W1 (chunk-combine fused into the u matmul)
W2_C = 128  # [128:256] W2
ONES_C = 256  # ones column
ONES1_C = 257  # row 0, [257:385] ones row
MPACK_W = 385

_CACHE = {}


def _build():
    from concourse import bacc, bass, mybir, tile

    f32 = mybir.dt.float32
    f32r = mybir.dt.float32r
    bf16 = mybir.dt.bfloat16
    nc = bacc.Bacc("TRN2", target_bir_lowering=False, debug=False)

    # ---- kernel I/O ----
    xin = nc.declare_dram_parameter("xpack", [40, NSUP * CH], bf16, isOutput=False)
    wpack_d = nc.declare_dram_parameter("wpack", [40, 256], bf16, isOutput=False)
    cpack_d = nc.declare_dram_parameter("cpack", [128, CPACK_W], f32, isOutput=False)
    mpack_d = nc.declare_dram_parameter("mpack", [128, MPACK_W], f32r, isOutput=False)
    out_d = nc.declare_dram_parameter("out", [OUT, B], f32, isOutput=True)

    AF = mybir.ActivationFunctionType
    ALU = mybir.AluOpType
    AX = mybir.AxisListType
    groups = [list(range(N_CORES))]

    with tile.TileContext(nc) as tc:
        with (
            tc.tile_pool(name="consts", bufs=1) as cp,
            tc.tile_pool(name="xyt", bufs=3) as xyp,
            tc.tile_pool(name="resp", bufs=2) as rp,
            tc.tile_pool(name="small", bufs=1) as sp,
            tc.tile_pool(name="dram", bufs=1, space="DRAM") as dp,
        ):
            zero_t = cp.tile([128, 1], f32)
            nc.gpsimd.memset(zero_t[:], 0.0)
            eps_t = cp.tile([128, 1], f32)
            nc.gpsimd.memset(eps_t[:], BN_EPS)
            zero_s = cp.tile([128, 512], bf16)
            nc.gpsimd.memset(zero_s[:], 0.0)

            # dummy exp: preloads the exp act table under the DMA prologue
            d0 = sp.tile([128, 1], bf16)
            nc.scalar.activation(d0[:], zero_t[:], AF.Exp, bias=zero_t[:], scale=1.0)

            # ---- batched constant loads ----
            wpack = cp.tile([40, 256], bf16)
            nc.sync.dma_start(out=wpack[:], in_=wpack_d[:])
            cpack = cp.tile([128, CPACK_W], f32)
            nc.sync.dma_start(out=cpack[:], in_=cpack_d[:])
            ebias_t = cpack[:, EB_C : EB_C + 1]
            w40hi = wpack[0:40, 0:128]
            w40lo = wpack[0:40, 128:256]

            s_all = sp.tile([128, BL], f32r)  # per-batch chunk-partial SLayer sums
            mpack = cp.tile([128, MPACK_W], f32r)

            # ================= SLayer phase =================
            with tc.tile_pool(name="pslayer", bufs=2, space="PSUM") as pp:
                offs = 0
                for bi, n in enumerate(BLOCK_SIZES):
                    w = n * CH
                    xyt = xyp.tile([128, 4 * CH], bf16, tag="xyt", name=f"xyt{bi}")
                    eng = (nc.sync, nc.gpsimd)[bi % 2]
                    eng.dma_start(
                        out=xyt[0:40, 0:w], in_=xin[:, offs * CH : offs * CH + w]
                    )
                    if bi == 3:
                        # mpack is only needed from the u matmul on; load it
                        # well after the pipeline ramp
                        nc.sync.dma_start(out=mpack[:], in_=mpack_d[:])
                    ps = pp.tile([128, 4 * CH], f32, tag="lg")
                    for j in range(n):
                        rhs = xyt[0:40, j * CH : (j + 1) * CH]
                        if bi == 0:
                            nc.tensor.matmul(
                                ps[:, j * CH : (j + 1) * CH], w40hi, rhs,
                                start=True, stop=True,
                            )
                            continue
                        nc.tensor.matmul(
                            ps[:, j * CH : (j + 1) * CH], w40hi, rhs,
                            start=True, stop=False,
                        )
                        nc.tensor.matmul(
                            ps[:, j * CH : (j + 1) * CH], w40lo, rhs,
                            start=False, stop=True,
                        )
                    resp = rp.tile([128, 4 * CH], bf16, tag="resp")
                    last_resp = resp
                    if n == 1 and bi >= 9:
                        # tail blocks: point-sum straight off the Exp's
                        # accumulator so the DVE tree isn't on the s_all path
                        with nc.allow_low_precision("f32r feature sums"):
                            nc.scalar.activation(
                                resp[:, 0:w], ps[:, 0:w], AF.Exp, bias=ebias_t,
                                scale=1.0, accum_out=s_all[:, offs : offs + 1],
                            )
                        offs += n
                        continue
                    nc.scalar.activation(
                        resp[:, 0:w], ps[:, 0:w], AF.Exp, bias=ebias_t, scale=1.0
                    )
                    # bf16 halving tree on DVE (4x mode), then grouped f32 reduce
                    r3 = resp[:, 0:w].rearrange("p (s c) -> p s c", s=n)
                    t1 = rp.tile([128, 4, 256], bf16, tag="t1")
                    nc.vector.tensor_tensor(
                        out=t1[:, 0:n, :], in0=r3[:, :, 0:256], in1=r3[:, :, 256:512],
                        op=ALU.add,
                    )
                    t2 = rp.tile([128, 4, 128], bf16, tag="t2")
                    nc.vector.tensor_tensor(
                        out=t2[:, 0:n, :], in0=t1[:, 0:n, 0:128], in1=t1[:, 0:n, 128:256],
                        op=ALU.add,
                    )
                    t3 = rp.tile([128, 4, 64], bf16, tag="t3")
                    nc.vector.tensor_tensor(
                        out=t3[:, 0:n, :], in0=t2[:, 0:n, 0:64], in1=t2[:, 0:n, 64:128],
                        op=ALU.add,
                    )
                    with nc.allow_low_precision("f32r feature sums"):
                        nc.vector.tensor_reduce(
                            out=s_all[:, offs : offs + n], in_=t3[:, 0:n, :],
                            axis=AX.X, op=ALU.add,
                        )
                    offs += n

            # dummy sqrt: switches the act table under the collective window
            # (reads the last exp output so it can't be hoisted before the exps)
            dsq = sp.tile([128, 1], f32)
            nc.scalar.activation(dsq[:], last_resp[:, 0:1], AF.Sqrt, bias=zero_t[:], scale=1.0)

            wr = mpack[:, WR_C : WR_C + HID]
            g1 = cpack[:, G1_C : G1_C + 1]
            b1 = cpack[:, B1_C : B1_C + 1]
            g2 = cpack[:, G2_C : G2_C + 1]
            b2 = cpack[:, B2_C : B2_C + 1]
            w2 = mpack[:, W2_C : W2_C + OUT]
            ones128 = mpack[:, ONES_C : ONES_C + 1]
            ones1 = mpack[0:1, ONES1_C : ONES1_C + 128]

            # ================= head phase =================
            with tc.tile_pool(name="pst", bufs=1, space="PSUM") as pt:
                # gather the raw chunk sums; the rmat chunk-combine is fused
                # into WR = rmat @ W1 host-side
                xb = dp.tile([128, BL], f32r, name="xb")
                nc.sync.dma_start(out=xb[:], in_=s_all[:])
                xg = dp.tile([N_CORES * 128, BL], f32r, name="xg", addr_space="Shared")
                cc = nc.gpsimd.collective_compute(
                    "AllGather",
                    ALU.bypass,
                    replica_groups=groups,
                    ins=[xb[:]],
                    outs=[xg[:]],
                )
                # express the same buffers as flat 1-D APs (lowered without
                # opt) so dim 0 carries the full extent
                cc.ins.ins[0] = nc.gpsimd.lower_ap(
                    xb[:].rearrange("r (c o) -> (r c) o", o=1), opt=False
                )
                cc.ins.outs[0] = nc.gpsimd.lower_ap(
                    xg[:].rearrange("r (c o) -> (r c) o", o=1), opt=False
                )

                # keep the PE p-state hot through the collective; the tiny
                # copy makes the dummies depend on the last-written s_all
                # column so they cannot preempt the head in the PE wait queue
                nc.vector.tensor_copy(zero_s[0:1, 0:1], s_all[0:1, BL - 1 : BL])
                dmt = pt.tile([128, 512], f32, tag="dummy")
                for _ in range(NDUMMY):
                    nc.tensor.matmul(dmt[:], zero_s[:, 0:128], zero_s[:])

                # gather -> xtf [128 chunkfeat, 256 batch] (batch = core-major)
                xtf = sp.tile([128, N_CORES, BL], f32r)
                nc.gpsimd.dma_start(
                    out=xtf[:], in_=xg[:].rearrange("(c f) b -> f c b", c=N_CORES)
                )
                xtf2 = xtf[:].rearrange("f c b -> f (c b)")

                # u = (rmat @ W1)^T s -> [128 hid, 256 batch]
                u_ps = pt.tile([HID, B], f32, tag="u")
                nc.tensor.matmul(u_ps[:], wr, xtf2)

                # BN1 stats (exact, global over the gathered batch)
                st1 = sp.tile([HID, 6], f32)
                nc.vector.bn_stats(st1[:], u_ps[:])
                mv1 = sp.tile([HID, 2], f32)
                nc.vector.bn_aggr(mv1[:], st1[:])
                sd1 = sp.tile([HID, 1], f32)
                nc.scalar.activation(sd1[:], mv1[:, 1:2], AF.Sqrt, bias=eps_t[:])
                rs1 = sp.tile([HID, 1], f32)
                nc.vector.reciprocal(rs1[:], sd1[:])
                a1 = sp.tile([HID, 1], f32)
                nc.vector.tensor_tensor(out=a1[:], in0=rs1[:], in1=g1, op=ALU.mult)
                nb1 = sp.tile([HID, 1], f32)
                nc.vector.tensor_tensor(out=nb1[:], in0=mv1[:, 0:1], in1=a1[:], op=ALU.mult)
                nc.vector.tensor_tensor(out=nb1[:], in0=b1, in1=nb1[:], op=ALU.subtract)

                # h = relu(a1 * u + nb1)
                h = rp.tile([HID, B], f32r, tag="h")
                nc.scalar.activation(h[:], u_ps[:], AF.Relu, bias=nb1[:], scale=a1[:])

                # y = h @ W2 -> [128 out, 256 batch]
                y_ps = pt.tile([OUT, B], f32, tag="y")
                nc.tensor.matmul(y_ps[:], w2, h[:])

                st2 = sp.tile([OUT, 6], f32)
                nc.vector.bn_stats(st2[:], y_ps[:])
                mv2 = sp.tile([OUT, 2], f32)
                nc.vector.bn_aggr(mv2[:], st2[:])
                sd2 = sp.tile([OUT, 1], f32)
                nc.scalar.activation(sd2[:], mv2[:, 1:2], AF.Sqrt, bias=eps_t[:])
                rs2 = sp.tile([OUT, 1], f32)
                nc.vector.reciprocal(rs2[:], sd2[:])
                a2 = sp.tile([OUT, 1], f32)
                nc.vector.tensor_tensor(out=a2[:], in0=rs2[:], in1=g2, op=ALU.mult)
                nb2 = sp.tile([OUT, 1], f32)
                nc.vector.tensor_tensor(out=nb2[:], in0=mv2[:, 0:1], in1=a2[:], op=ALU.mult)
                nc.vector.tensor_tensor(out=nb2[:], in0=b2, in1=nb2[:], op=ALU.subtract)

                # y_bn = a2*y + nb2 ; L2-normalize columns. ysq is computed
                # first, fused from y_ps, so the q/sdq chain overlaps y_bn.
                ysq = rp.tile([OUT, B], f32r, tag="ysq")
                nc.scalar.activation(ysq[:], y_ps[:], AF.Square, bias=nb2[:], scale=a2[:])
                q_ps = pt.tile([1, B], f32, tag="q")
                nc.tensor.matmul(q_ps[:], ones128, ysq[:])
                sdq = sp.tile([1, B], f32)
                nc.scalar.activation(sdq[:], q_ps[:], AF.Sqrt, bias=zero_t[0:1, :])
                rq = sp.tile([1, B], f32r)
                with nc.allow_low_precision("f32r norm scale"):
                    nc.vector.reciprocal(rq[:], sdq[:])
                y_bn = rp.tile([OUT, B], f32, tag="ybn")
                nc.scalar.activation(y_bn[:], y_ps[:], AF.Identity, bias=nb2[:], scale=a2[:])
                rnb_ps = pt.tile([128, B], f32, tag="rnb")
                nc.tensor.matmul(rnb_ps[:], ones1, rq[:])
                # final scale + store, split in column halves across two DMA
                # queues so the second half's store starts earlier
                out_sb = rp.tile([OUT, B], f32, tag="osb")
                HB = B // 2
                nc.vector.tensor_tensor(
                    out=out_sb[:, 0:HB], in0=y_bn[:, 0:HB], in1=rnb_ps[:, 0:HB],
                    op=ALU.mult,
                )
                nc.sync.dma_start(out=out_d[:, 0:HB], in_=out_sb[:, 0:HB])
                nc.vector.tensor_tensor(
                    out=out_sb[:, HB:B], in0=y_bn[:, HB:B], in1=rnb_ps[:, HB:B],
                    op=ALU.mult,
                )
                nc.scalar.dma_start(out=out_d[:, HB:B], in_=out_sb[:, HB:B])

    nc.finalize()
    return nc


def _get_nc():
    if "nc" not in _CACHE:
        _CACHE["nc"] = _build()
    return _CACHE["nc"]


def _softplus(x):
    return np.logaddexp(0.0, x)


def _prep_weights(centers0, log_sharp0, centers1, log_sharp1, W1, W2,
                  gamma1, beta1, gamma2, beta2):
    """Host-side packing of the tiny SLayer/head params."""
    import ml_dtypes

    s0 = _softplus(log_sharp0.astype(np.float64)) + 1e-6  # [E,2]
    s1 = _softplus(log_sharp1.astype(np.float64)) + 1e-6
    c0 = centers0.astype(np.float64)
    c1 = centers1.astype(np.float64)

    # 40 rhs rows per batch: j=0..7 x of group j (0..3 h0 chunks, 4..7 h1),
    # 8..15 y, 16..23 x^2, 24..31 y^2, 32..39 mask. outputs 16G+e.
    w1b = np.zeros((40, 128), np.float64)
    ebias = np.zeros(128, np.float64)
    for G in range(8):
        me = 16 * G + np.arange(E)
        sx, sy = (s0[:, 0], s0[:, 1]) if G < 4 else (s1[:, 0], s1[:, 1])
        cx, cy = (c0[:, 0], c0[:, 1]) if G < 4 else (c1[:, 0], c1[:, 1])
        w1b[G, me] = 2.0 * sx * cx
        w1b[8 + G, me] = 2.0 * sy * cy
        w1b[16 + G, me] = -sx
        w1b[24 + G, me] = -sy
        w1b[32 + G, me] = MASK_W
        ebias[me] = -(sx * cx**2 + sy * cy**2)

    # weight hi/lo residual: the kernel runs two accumulating matmuls
    # (w_hi then w_lo) against the same bf16 data rows
    w_hi = w1b.astype(ml_dtypes.bfloat16).astype(np.float64)
    w_lo = w1b - w_hi
    w40 = np.zeros((40, 256), np.float64)
    w40[:, 0:128] = w_hi
    w40[:, 128:256] = w_lo

    rmat = np.zeros((128, 32), np.float32)
    for g in range(4):
        for e in range(E):
            rmat[16 * g + e, e] = 1.0
            rmat[64 + 16 * g + e, 16 + e] = 1.0

    cpack = np.zeros((128, CPACK_W), np.float32)
    cpack[:, EB_C] = ebias.astype(np.float32)
    cpack[:, G1_C] = np.asarray(gamma1, np.float32)
    cpack[:, B1_C] = np.asarray(beta1, np.float32)
    cpack[:, G2_C] = np.asarray(gamma2, np.float32)
    cpack[:, B2_C] = np.asarray(beta2, np.float32)

    mpack = np.zeros((128, MPACK_W), np.float32)
    mpack[:, WR_C : WR_C + HID] = (
        rmat.astype(np.float64) @ np.asarray(W1, np.float64)
    ).astype(np.float32)
    mpack[:, W2_C : W2_C + OUT] = np.asarray(W2, np.float32)
    mpack[:, ONES_C] = 1.0
    mpack[0, ONES1_C : ONES1_C + 128] = 1.0

    return w40.astype(ml_dtypes.bfloat16), cpack, mpack


def _pack_core(bc0, bc1, c0, c1):
    """Pack one core's barcodes into a [128, NSUP*CH] bf16 rhs supertile row."""
    import ml_dtypes

    BLc = bc0.shape[0]
    blk = np.zeros((BLc, 128, CH), np.float32)  # per-batch rows at base 0
    pidx = np.arange(P, dtype=np.float32).reshape(NCH, CH)  # global point index
    for h, (bc, cnt) in enumerate(((bc0, c0), (bc1, c1))):
        xy = bc.reshape(BLc, NCH, CH, 2)
        g0 = 4 * h
        blk[:, g0 : g0 + 4] = xy[..., 0]
        blk[:, 8 + g0 : 12 + g0] = xy[..., 1]
        blk[:, 16 + g0 : 20 + g0] = xy[..., 0] ** 2
        blk[:, 24 + g0 : 28 + g0] = xy[..., 1] ** 2
        mask = (pidx[None] >= cnt[:, None, None]).astype(np.float32)
        blk[:, 32 + g0 : 36 + g0] = mask
    # one batch per supertile, 40 bf16 data rows
    sup = blk[:, 0:40].astype(ml_dtypes.bfloat16)
    # supertile-major along the free dim: [40, NSUP*CH]
    return np.ascontiguousarray(sup.transpose(1, 0, 2).reshape(40, BLc * CH))


def kernel(
    barcode_h0,
    barcode_h0_count,
    barcode_h1,
    barcode_h1_count,
    centers0,
    log_sharp0,
    centers1,
    log_sharp1,
    W1,
    gamma1,
    beta1,
    W2,
    gamma2,
    beta2,
):
    from concourse.bass_utils import run_bass_kernel_spmd

    nc = _get_nc()
    w40, cpack, mpack = _prep_weights(
        centers0, log_sharp0, centers1, log_sharp1, W1, W2,
        gamma1, beta1, gamma2, beta2,
    )

    bc0 = np.ascontiguousarray(barcode_h0, dtype=np.float32)
    bc1 = np.ascontiguousarray(barcode_h1, dtype=np.float32)
    c0 = np.asarray(barcode_h0_count).astype(np.float32)
    c1 = np.asarray(barcode_h1_count).astype(np.float32)

    in_maps = []
    for c in range(N_CORES):
        sl = slice(c * BL, (c + 1) * BL)
        in_maps.append(
            {
                "xpack": _pack_core(bc0[sl], bc1[sl], c0[sl], c1[sl]),
                "wpack": w40,
                "cpack": cpack,
                "mpack": mpack,
            }
        )

    _CACHE["in_maps"] = in_maps
    res = run_bass_kernel_spmd(nc, in_maps, core_ids=list(range(N_CORES)))
    out = res.results[0]["out"]  # [OUT, B]
    return np.ascontiguousarray(out.T).astype(np.float32)


# revision 22
# speedup vs baseline: 1.0047x; 1.0047x over previous
"""Trainium2 Bass kernel for nn_BarcodeSLayerEncoder.

Design (8 NeuronCores, pure data-parallel over batch):
  - Each core gets B/8 = 32 batches of both barcode tensors.
  - SLayer logits via TensorE matmuls (K=120, 512 cols per batch):
      logit[16g+e, p] = 2*s_x*c_x[e]*x - s_x*x^2 + 2*s_y*c_y[e]*y - s_y*y^2
                        - 100*maskflag(g,p)          (g = 4 h0-chunks + 4 h1-chunks)
    with the per-center constant -s*||c||^2 folded into the ScalarE Exp bias.
    Supertiles are processed in blocks of up to 4 batches so one wide Exp
    activation covers [128, 2048] (amortizes the ACT access latency).
  - Point sums: 3 bf16 TensorTensor halvings on DVE (4x perf mode) then one
    grouped f32 tensor_reduce -> s_all f32 (bf16 rounding stays ~1e-3 overall).
  - Features x2 [32,32] f32 -> AllGather -> every core redundantly computes
    the tiny head with exact global BN stats (bn_stats/bn_aggr for mean+var).
    Head matmuls use float32r (full precision, 1 cycle/row at >=256 cols).
  - Act tables: a dummy Exp at t=0 preloads the exp table under the DMA
    prologue; a dummy Sqrt right after the last Exp switches to the sqrt
    table under the collective. Dummy matmuls keep the PE p-state at full
    clock through the collective so the head matmuls run at 2.4 GHz.
"""

import sys

sys.path.insert(0, "/opt/trn_rl_repo")

import numpy as np

N_CORES = 8
B, P, E, D = 256, 2048, 16, 2
BL = B // N_CORES  # 32 local batches per core
HID, OUT = 128, 128
BN_EPS = 1e-5
CH = 512  # point chunk size (one PSUM bank of f32)
NCH = P // CH  # 4 chunks per homology
NSUP = BL  # one batch per supertile
MASK_W = -100.0
BLOCK_SIZES = [1, 2, 3, 4, 4, 4, 4, 4, 3, 1, 1, 1]  # supertiles per exp block
NDUMMY = 92  # PE keep-warm matmuls spanning the collective window

# cpack (f32) column layout: ebias, g1, b1, g2, b2
EB_C = 0
G1_C, B1_C, G2_C, B2_C = 1, 2, 3, 4
CPACK_W = 5
# mpack (f32r) column layout: matmul weights for the head
WR_C = 0  # [0:128] WR = rmat @ W1 (chunk-combine fused into the u matmul)
W2_C = 128  # [128:256] W2
ONES_C = 256  # ones column
ONES1_C = 257  # row 0, [257:385] ones row
MPACK_W = 385

_CACHE = {}


def _build():
    from concourse import bacc, bass, mybir, tile

    f32 = mybir.dt.float32
    f32r = mybir.dt.float32r
    bf16 = mybir.dt.bfloat16
    nc = bacc.Bacc("TRN2", target_bir_lowering=False, debug=False)

    # ---- kernel I/O ----
    xin = nc.declare_dram_parameter("xpack", [40, NSUP * CH], bf16, isOutput=False)
    wpack_d = nc.declare_dram_parameter("wpack", [40, 256], bf16, isOutput=False)
    cpack_d = nc.declare_dram_parameter("cpack", [128, CPACK_W], f32, isOutput=False)
    mpack_d = nc.declare_dram_parameter("mpack", [128, MPACK_W], f32r, isOutput=False)
    out_d = nc.declare_dram_parameter("out", [OUT, B], f32, isOutput=True)

    AF = mybir.ActivationFunctionType
    ALU = mybir.AluOpType
    AX = mybir.AxisListType
    groups = [list(range(N_CORES))]

    with tile.TileContext(nc) as tc:
        with (
            tc.tile_pool(name="consts", bufs=1) as cp,
            tc.tile_pool(name="xyt", bufs=3) as xyp,
            tc.tile_pool(name="resp", bufs=2) as rp,
            tc.tile_pool(name="small", bufs=1) as sp,
            tc.tile_pool(name="dram", bufs=1, space="DRAM") as dp,
        ):
            zero_t = cp.tile([128, 1], f32)
            nc.gpsimd.memset(zero_t[:], 0.0)
            eps_t = cp.tile([128, 1], f32)
            nc.gpsimd.memset(eps_t[:], BN_EPS)
            zero_s = cp.tile([128, 512], bf16)
            nc.gpsimd.memset(zero_s[:], 0.0)

            # dummy exp: preloads the exp act table under the DMA prologue
            d0 = sp.tile([128, 1], bf16)
            nc.scalar.activation(d0[:], zero_t[:], AF.Exp, bias=zero_t[:], scale=1.0)

            # ---- batched constant loads ----
            wpack = cp.tile([40, 256], bf16)
            nc.sync.dma_start(out=wpack[:], in_=wpack_d[:])
            cpack = cp.tile([128, CPACK_W], f32)
            nc.sync.dma_start(out=cpack[:], in_=cpack_d[:])
            ebias_t = cpack[:, EB_C : EB_C + 1]
            w40hi = wpack[0:40, 0:128]
            w40lo = wpack[0:40, 128:256]

            s_all = sp.tile([128, BL], f32r)  # per-batch chunk-partial SLayer sums
            mpack = cp.tile([128, MPACK_W], f32r)

            # ================= SLayer phase =================
            with tc.tile_pool(name="pslayer", bufs=2, space="PSUM") as pp:
                offs = 0
                for bi, n in enumerate(BLOCK_SIZES):
                    w = n * CH
                    xyt = xyp.tile([128, 4 * CH], bf16, tag="xyt", name=f"xyt{bi}")
                    eng = (nc.sync, nc.gpsimd)[bi % 2]
                    eng.dma_start(
                        out=xyt[0:40, 0:w], in_=xin[:, offs * CH : offs * CH + w]
                    )
                    if bi == 3:
                        # mpack is only needed from the u matmul on; load it
                        # well after the pipeline ramp
                        nc.sync.dma_start(out=mpack[:], in_=mpack_d[:])
                    ps = pp.tile([128, 4 * CH], f32, tag="lg")
                    for j in range(n):
                        rhs = xyt[0:40, j * CH : (j + 1) * CH]
                        if bi == 0:
                            nc.tensor.matmul(
                                ps[:, j * CH : (j + 1) * CH], w40hi, rhs,
                                start=True, stop=True,
                            )
                            continue
                        nc.tensor.matmul(
                            ps[:, j * CH : (j + 1) * CH], w40hi, rhs,
                            start=True, stop=False,
                        )
                        nc.tensor.matmul(
                            ps[:, j * CH : (j + 1) * CH], w40lo, rhs,
                            start=False, stop=True,
                        )
                    resp = rp.tile([128, 4 * CH], bf16, tag="resp")
                    last_resp = resp
                    if n == 1 and bi >= 9:
                        # tail blocks: point-sum straight off the Exp's
                        # accumulator so the DVE tree isn't on the s_all path
                        with nc.allow_low_precision("f32r feature sums"):
                            nc.scalar.activation(
                                resp[:, 0:w], ps[:, 0:w], AF.Exp, bias=ebias_t,
                                scale=1.0, accum_out=s_all[:, offs : offs + 1],
                            )
                        offs += n
                        continue
                    nc.scalar.activation(
                        resp[:, 0:w], ps[:, 0:w], AF.Exp, bias=ebias_t, scale=1.0
                    )
                    # bf16 halving tree on DVE (4x mode), then grouped f32 reduce
                    r3 = resp[:, 0:w].rearrange("p (s c) -> p s c", s=n)
                    t1 = rp.tile([128, 4, 256], bf16, tag="t1")
                    nc.vector.tensor_tensor(
                        out=t1[:, 0:n, :], in0=r3[:, :, 0:256], in1=r3[:, :, 256:512],
                        op=ALU.add,
                    )
                    t2 = rp.tile([128, 4, 128], bf16, tag="t2")
                    nc.vector.tensor_tensor(
                        out=t2[:, 0:n, :], in0=t1[:, 0:n, 0:128], in1=t1[:, 0:n, 128:256],
                        op=ALU.add,
                    )
                    t3 = rp.tile([128, 4, 64], bf16, tag="t3")
                    nc.vector.tensor_tensor(
                        out=t3[:, 0:n, :], in0=t2[:, 0:n, 0:64], in1=t2[:, 0:n, 64:128],
                        op=ALU.add,
                    )
                    with nc.allow_low_precision("f32r feature sums"):
                        nc.vector.tensor_reduce(
                            out=s_all[:, offs : offs + n], in_=t3[:, 0:n, :],
                            axis=AX.X, op=ALU.add,
                        )
                    offs += n

            # dummy sqrt: switches the act table under the collective window
            # (reads the last exp output so it can't be hoisted before the exps)
            dsq = sp.tile([128, 1], f32)
            nc.scalar.activation(dsq[:], last_resp[:, 0:1], AF.Sqrt, bias=zero_t[:], scale=1.0)

            wr = mpack[:, WR_C : WR_C + HID]
            g1 = cpack[:, G1_C : G1_C + 1]
            b1 = cpack[:, B1_C : B1_C + 1]
            g2 = cpack[:, G2_C : G2_C + 1]
            b2 = cpack[:, B2_C : B2_C + 1]
            w2 = mpack[:, W2_C : W2_C + OUT]
            ones128 = mpack[:, ONES_C : ONES_C + 1]
            ones1 = mpack[0:1, ONES1_C : ONES1_C + 128]

            # ================= head phase =================
            with tc.tile_pool(name="pst", bufs=1, space="PSUM") as pt:
                # gather the raw chunk sums; the rmat chunk-combine is fused
                # into WR = rmat @ W1 host-side
                xb = dp.tile([128, BL], f32r, name="xb")
                nc.sync.dma_start(out=xb[:], in_=s_all[:])
                xg = dp.tile([N_CORES * 128, BL], f32r, name="xg", addr_space="Shared")
                cc = nc.gpsimd.collective_compute(
                    "AllGather",
                    ALU.bypass,
                    replica_groups=groups,
                    ins=[xb[:]],
                    outs=[xg[:]],
                )
                # express the same buffers as flat 1-D APs (lowered without
                # opt) so dim 0 carries the full extent
                cc.ins.ins[0] = nc.gpsimd.lower_ap(
                    xb[:].rearrange("r (c o) -> (r c) o", o=1), opt=False
                )
                cc.ins.outs[0] = nc.gpsimd.lower_ap(
                    xg[:].rearrange("r (c o) -> (r c) o", o=1), opt=False
                )

                # keep the PE p-state hot through the collective; the tiny
                # copy makes the dummies depend on the last-written s_all
                # column so they cannot preempt the head in the PE wait queue
                nc.vector.tensor_copy(zero_s[0:1, 0:1], s_all[0:1, BL - 1 : BL])
                dmt = pt.tile([128, 512], f32, tag="dummy")
                for _ in range(NDUMMY):
                    nc.tensor.matmul(dmt[:], zero_s[:, 0:128], zero_s[:])

                # gather -> xtf [128 chunkfeat, 256 batch] (batch = core-major)
                xtf = sp.tile([128, N_CORES, BL], f32r)
                nc.gpsimd.dma_start(
                    out=xtf[:], in_=xg[:].rearrange("(c f) b -> f c b", c=N_CORES)
                )
                xtf2 = xtf[:].rearrange("f c b -> f (c b)")

                # u = (rmat @ W1)^T s -> [128 hid, 256 batch]
                u_ps = pt.tile([HID, B], f32, tag="u")
                nc.tensor.matmul(u_ps[:], wr, xtf2)

                # BN1 stats (exact, global over the gathered batch)
                st1 = sp.tile([HID, 6], f32)
                nc.vector.bn_stats(st1[:], u_ps[:])
                mv1 = sp.tile([HID, 2], f32)
                nc.vector.bn_aggr(mv1[:], st1[:])
                sd1 = sp.tile([HID, 1], f32)
                nc.scalar.activation(sd1[:], mv1[:, 1:2], AF.Sqrt, bias=eps_t[:])
                rs1 = sp.tile([HID, 1], f32)
                nc.vector.reciprocal(rs1[:], sd1[:])
                a1 = sp.tile([HID, 1], f32)
                nc.vector.tensor_tensor(out=a1[:], in0=rs1[:], in1=g1, op=ALU.mult)
                nb1 = sp.tile([HID, 1], f32)
                nc.vector.tensor_tensor(out=nb1[:], in0=mv1[:, 0:1], in1=a1[:], op=ALU.mult)
                nc.vector.tensor_tensor(out=nb1[:], in0=b1, in1=nb1[:], op=ALU.subtract)

                # h = relu(a1 * u + nb1)
                h = rp.tile([HID, B], f32r, tag="h")
                nc.scalar.activation(h[:], u_ps[:], AF.Relu, bias=nb1[:], scale=a1[:])

                # y = h @ W2 -> [128 out, 256 batch]
                y_ps = pt.tile([OUT, B], f32, tag="y")
                nc.tensor.matmul(y_ps[:], w2, h[:])

                st2 = sp.tile([OUT, 6], f32)
                nc.vector.bn_stats(st2[:], y_ps[:])
                mv2 = sp.tile([OUT, 2], f32)
                nc.vector.bn_aggr(mv2[:], st2[:])
                sd2 = sp.tile([OUT, 1], f32)
                nc.scalar.activation(sd2[:], mv2[:, 1:2], AF.Sqrt, bias=eps_t[:])
                rs2 = sp.tile([OUT, 1], f32)
                nc.vector.reciprocal(rs2[:], sd2[:])
                a2 = sp.tile([OUT, 1], f32)
                nc.vector.tensor_tensor(out=a2[:], in0=rs2[:], in1=g2, op=ALU.mult)
                nb2 = sp.tile([OUT, 1], f32)
                nc.vector.tensor_tensor(out=nb2[:], in0=mv2[:, 0:1], in1=a2[:], op=ALU.mult)
                nc.vector.tensor_tensor(out=nb2[:], in0=b2, in1=nb2[:], op=ALU.subtract)

                # y_bn = a2*y + nb2 ; L2-normalize columns. ysq is computed
                # first, fused from y_ps, so the q/sdq chain overlaps y_bn.
                ysq = rp.tile([OUT, B], f32r, tag="ysq")
                nc.scalar.activation(ysq[:], y_ps[:], AF.Square, bias=nb2[:], scale=a2[:])
                q_ps = pt.tile([1, B], f32, tag="q")
                nc.tensor.matmul(q_ps[:], ones128, ysq[:])
                sdq = sp.tile([1, B], f32)
                nc.scalar.activation(sdq[:], q_ps[:], AF.Sqrt, bias=zero_t[0:1, :])
                rq = sp.tile([1, B], f32r)
                with nc.allow_low_precision("f32r norm scale"):
                    nc.vector.reciprocal(rq[:], sdq[:])
                y_bn = rp.tile([OUT, B], f32, tag="ybn")
                nc.scalar.activation(y_bn[:], y_ps[:], AF.Identity, bias=nb2[:], scale=a2[:])
                rnb_ps = pt.tile([128, B], f32, tag="rnb")
                nc.tensor.matmul(rnb_ps[:], ones1, rq[:])
                out_sb = rp.tile([OUT, B], f32, tag="osb")
                nc.vector.tensor_tensor(
                    out=out_sb[:], in0=y_bn[:], in1=rnb_ps[:], op=ALU.mult
                )
                nc.sync.dma_start(out=out_d[:], in_=out_sb[:])

    nc.finalize()
    return nc


def _get_nc():
    if "nc" not in _CACHE:
        _CACHE["nc"] = _build()
    return _CACHE["nc"]


def _softplus(x):
    return np.logaddexp(0.0, x)


def _prep_weights(centers0, log_sharp0, centers1, log_sharp1, W1, W2,
                  gamma1, beta1, gamma2, beta2):
    """Host-side packing of the tiny SLayer/head params."""
    import ml_dtypes

    s0 = _softplus(log_sharp0.astype(np.float64)) + 1e-6  # [E,2]
    s1 = _softplus(log_sharp1.astype(np.float64)) + 1e-6
    c0 = centers0.astype(np.float64)
    c1 = centers1.astype(np.float64)

    # 40 rhs rows per batch: j=0..7 x of group j (0..3 h0 chunks, 4..7 h1),
    # 8..15 y, 16..23 x^2, 24..31 y^2, 32..39 mask. outputs 16G+e.
    w1b = np.zeros((40, 128), np.float64)
    ebias = np.zeros(128, np.float64)
    for G in range(8):
        me = 16 * G + np.arange(E)
        sx, sy = (s0[:, 0], s0[:, 1]) if G < 4 else (s1[:, 0], s1[:, 1])
        cx, cy = (c0[:, 0], c0[:, 1]) if G < 4 else (c1[:, 0], c1[:, 1])
        w1b[G, me] = 2.0 * sx * cx
        w1b[8 + G, me] = 2.0 * sy * cy
        w1b[16 + G, me] = -sx
        w1b[24 + G, me] = -sy
        w1b[32 + G, me] = MASK_W
        ebias[me] = -(sx * cx**2 + sy * cy**2)

    # weight hi/lo residual: the kernel runs two accumulating matmuls
    # (w_hi then w_lo) against the same bf16 data rows
    w_hi = w1b.astype(ml_dtypes.bfloat16).astype(np.float64)
    w_lo = w1b - w_hi
    w40 = np.zeros((40, 256), np.float64)
    w40[:, 0:128] = w_hi
    w40[:, 128:256] = w_lo

    rmat = np.zeros((128, 32), np.float32)
    for g in range(4):
        for e in range(E):
            rmat[16 * g + e, e] = 1.0
            rmat[64 + 16 * g + e, 16 + e] = 1.0

    cpack = np.zeros((128, CPACK_W), np.float32)
    cpack[:, EB_C] = ebias.astype(np.float32)
    cpack[:, G1_C] = np.asarray(gamma1, np.float32)
    cpack[:, B1_C] = np.asarray(beta1, np.float32)
    cpack[:, G2_C] = np.asarray(gamma2, np.float32)
    cpack[:, B2_C] = np.asarray(beta2, np.float32)

    mpack = np.zeros((128, MPACK_W), np.float32)
    mpack[:, WR_C : WR_C + HID] = (
        rmat.astype(np.float64) @ np.asarray(W1, np.float64)
    ).astype(np.float32)
    mpack[:, W2_C : W2_C + OUT] = np.asarray(W2, np.float32)
    mpack[:, ONES_C] = 1.0
    mpack[0, ONES1_C : ONES1_C + 128] = 1.0

    return w40.astype(ml_dtypes.bfloat16), cpack, mpack


def _pack_core(bc0, bc1, c0, c1):
    """Pack one core's barcodes into a [128, NSUP*CH] bf16 rhs supertile row."""
    import ml_dtypes

    BLc = bc0.shape[0]
    blk = np.zeros((BLc, 128, CH), np.float32)  # per-batch rows at base 0
    pidx = np.arange(P, dtype=np.float32).reshape(NCH, CH)  # global point index
    for h, (bc, cnt) in enumerate(((bc0, c0), (bc1, c1))):
        xy = bc.reshape(BLc, NCH, CH, 2)
        g0 = 4 * h
        blk[:, g0 : g0 + 4] = xy[..., 0]
        blk[:, 8 + g0 : 12 + g0] = xy[..., 1]
        blk[:, 16 + g0 : 20 + g0] = xy[..., 0] ** 2
        blk[:, 24 + g0 : 28 + g0] = xy[..., 1] ** 2
        mask = (pidx[None] >= cnt[:, None, None]).astype(np.float32)
        blk[:, 32 + g0 : 36 + g0] = mask
    # one batch per supertile, 40 bf16 data rows
    sup = blk[:, 0:40].astype(ml_dtypes.bfloat16)
    # supertile-major along the free dim: [40, NSUP*CH]
    return np.ascontiguousarray(sup.transpose(1, 0, 2).reshape(40, BLc * CH))


def kernel(
    barcode_h0,
    barcode_h0_count,
    barcode_h1,
    barcode_h1_count,
    centers0,
    log_sharp0,
    centers1,
    log_sharp1,
    W1,
    gamma1,
    beta1,
    W2,
    gamma2,
    beta2,
):
    from concourse.bass_utils import run_bass_kernel_spmd

    nc = _get_nc()
    w40, cpack, mpack = _prep_weights(
        centers0, log_sharp0, centers1, log_sharp1, W1, W2,
        gamma1, beta1, gamma2, beta2,
    )

    bc0 = np.ascontiguousarray(barcode_h0, dtype=np.float32)
    bc1 = np.ascontiguousarray(barcode_h1, dtype=np.float32)
    c0 = np.asarray(barcode_h0_count).astype(np.float32)
    c1 = np.asarray(barcode_h1_count).astype(np.float32)

    in_maps = []
    for c in range(N_CORES):
        sl = slice(c * BL, (c + 1) * BL)
        in_maps.append(
            {
                "xpack": _pack_core(bc0[sl], bc1[sl], c0[sl], c1[sl]),
                "wpack": w40,
                "cpack": cpack,
                "mpack": mpack,
            }
        )

    _CACHE["in_maps"] = in_maps
    res = run_bass_kernel_spmd(nc, in_maps, core_ids=list(range(N_CORES)))
    out = res.results[0]["out"]  # [OUT, B]
    return np.ascontiguousarray(out.T).astype(np.float32)


# revision 23
# speedup vs baseline: 1.0184x; 1.0136x over previous
"""Trainium2 Bass kernel for nn_BarcodeSLayerEncoder.

Design (8 NeuronCores, pure data-parallel over batch):
  - Each core gets B/8 = 32 batches of both barcode tensors.
  - SLayer logits via TensorE matmuls (K=120, 512 cols per batch):
      logit[16g+e, p] = 2*s_x*c_x[e]*x - s_x*x^2 + 2*s_y*c_y[e]*y - s_y*y^2
                        - 100*maskflag(g,p)          (g = 4 h0-chunks + 4 h1-chunks)
    with the per-center constant -s*||c||^2 folded into the ScalarE Exp bias.
    Supertiles are processed in blocks of up to 4 batches so one wide Exp
    activation covers [128, 2048] (amortizes the ACT access latency).
  - Point sums: 3 bf16 TensorTensor halvings on DVE (4x perf mode) then one
    grouped f32 tensor_reduce -> s_all f32 (bf16 rounding stays ~1e-3 overall).
  - Features x2 [32,32] f32 -> AllGather -> every core redundantly computes
    the tiny head with exact global BN stats (bn_stats/bn_aggr for mean+var).
    Head matmuls use float32r (full precision, 1 cycle/row at >=256 cols).
  - Act tables: a dummy Exp at t=0 preloads the exp table under the DMA
    prologue; a dummy Sqrt right after the last Exp switches to the sqrt
    table under the collective. Dummy matmuls keep the PE p-state at full
    clock through the collective so the head matmuls run at 2.4 GHz.
"""

import sys

sys.path.insert(0, "/opt/trn_rl_repo")

import numpy as np

N_CORES = 8
B, P, E, D = 256, 2048, 16, 2
BL = B // N_CORES  # 32 local batches per core
HID, OUT = 128, 128
BN_EPS = 1e-5
CH = 512  # point chunk size (one PSUM bank of f32)
NCH = P // CH  # 4 chunks per homology
NSUP = BL  # one batch per supertile
MASK_W = -100.0
BLOCK_SIZES = [1, 2, 3, 4, 4, 4, 4, 4, 3, 1, 1, 1]  # supertiles per exp block
NDUMMY = 89  # PE keep-warm matmuls spanning the collective window

# cpack (f32) column layout: ebias, g1, b1, g2, b2
EB_C = 0
G1_C, B1_C, G2_C, B2_C = 1, 2, 3, 4
CPACK_W = 5
# mpack (f32r) column layout: matmul weights for the head
WR_C = 0  # [0:128] WR = rmat @ W1 (chunk-combine fused into the u matmul)
W2_C = 128  # [128:256] W2
ONES_C = 256  # ones column
ONES1_C = 257  # row 0, [257:385] ones row
MPACK_W = 385

_CACHE = {}


def _build():
    from concourse import bacc, bass, mybir, tile

    f32 = mybir.dt.float32
    f32r = mybir.dt.float32r
    bf16 = mybir.dt.bfloat16
    nc = bacc.Bacc("TRN2", target_bir_lowering=False, debug=False)

    # ---- kernel I/O ----
    xin = nc.declare_dram_parameter("xpack", [40, NSUP * CH], bf16, isOutput=False)
    wpack_d = nc.declare_dram_parameter("wpack", [40, 256], bf16, isOutput=False)
    cpack_d = nc.declare_dram_parameter("cpack", [128, CPACK_W], f32, isOutput=False)
    mpack_d = nc.declare_dram_parameter("mpack", [128, MPACK_W], f32r, isOutput=False)
    out_d = nc.declare_dram_parameter("out", [OUT, B], f32, isOutput=True)

    AF = mybir.ActivationFunctionType
    ALU = mybir.AluOpType
    AX = mybir.AxisListType
    groups = [list(range(N_CORES))]

    with tile.TileContext(nc) as tc:
        with (
            tc.tile_pool(name="consts", bufs=1) as cp,
            tc.tile_pool(name="xyt", bufs=3) as xyp,
            tc.tile_pool(name="resp", bufs=2) as rp,
            tc.tile_pool(name="small", bufs=1) as sp,
            tc.tile_pool(name="dram", bufs=1, space="DRAM") as dp,
        ):
            zero_t = cp.tile([128, 1], f32)
            nc.gpsimd.memset(zero_t[:], 0.0)
            eps_t = cp.tile([128, 1], f32)
            nc.gpsimd.memset(eps_t[:], BN_EPS)
            zero_s = cp.tile([128, 512], bf16)
            nc.gpsimd.memset(zero_s[:], 0.0)

            # dummy exp: preloads the exp act table under the DMA prologue
            d0 = sp.tile([128, 1], bf16)
            nc.scalar.activation(d0[:], zero_t[:], AF.Exp, bias=zero_t[:], scale=1.0)

            # ---- batched constant loads ----
            wpack = cp.tile([40, 256], bf16)
            nc.sync.dma_start(out=wpack[:], in_=wpack_d[:])
            cpack = cp.tile([128, CPACK_W], f32)
            nc.sync.dma_start(out=cpack[:], in_=cpack_d[:])
            ebias_t = cpack[:, EB_C : EB_C + 1]
            w40hi = wpack[0:40, 0:128]
            w40lo = wpack[0:40, 128:256]

            s_all = sp.tile([128, BL], f32r)  # per-batch chunk-partial SLayer sums
            mpack = cp.tile([128, MPACK_W], f32r)

            # ================= SLayer phase =================
            with tc.tile_pool(name="pslayer", bufs=2, space="PSUM") as pp:
                offs = 0
                for bi, n in enumerate(BLOCK_SIZES):
                    w = n * CH
                    xyt = xyp.tile([128, 4 * CH], bf16, tag="xyt", name=f"xyt{bi}")
                    eng = (nc.sync, nc.gpsimd)[bi % 2]
                    eng.dma_start(
                        out=xyt[0:40, 0:w], in_=xin[:, offs * CH : offs * CH + w]
                    )
                    if bi == 3:
                        # mpack is only needed from the u matmul on; load it
                        # well after the pipeline ramp
                        nc.sync.dma_start(out=mpack[:], in_=mpack_d[:])
                    ps = pp.tile([128, 4 * CH], f32, tag="lg")
                    for j in range(n):
                        rhs = xyt[0:40, j * CH : (j + 1) * CH]
                        if bi == 0:
                            nc.tensor.matmul(
                                ps[:, j * CH : (j + 1) * CH], w40hi, rhs,
                                start=True, stop=True,
                            )
                            continue
                        nc.tensor.matmul(
                            ps[:, j * CH : (j + 1) * CH], w40hi, rhs,
                            start=True, stop=False,
                        )
                        nc.tensor.matmul(
                            ps[:, j * CH : (j + 1) * CH], w40lo, rhs,
                            start=False, stop=True,
                        )
                    resp = rp.tile([128, 4 * CH], bf16, tag="resp")
                    last_resp = resp
                    if n == 1 and bi >= 9:
                        # tail blocks: point-sum straight off the Exp's
                        # accumulator so the DVE tree isn't on the s_all path
                        with nc.allow_low_precision("f32r feature sums"):
                            nc.scalar.activation(
                                resp[:, 0:w], ps[:, 0:w], AF.Exp, bias=ebias_t,
                                scale=1.0, accum_out=s_all[:, offs : offs + 1],
                            )
                        offs += n
                        continue
                    nc.scalar.activation(
                        resp[:, 0:w], ps[:, 0:w], AF.Exp, bias=ebias_t, scale=1.0
                    )
                    # bf16 halving tree on DVE (4x mode), then grouped f32 reduce
                    r3 = resp[:, 0:w].rearrange("p (s c) -> p s c", s=n)
                    t1 = rp.tile([128, 4, 256], bf16, tag="t1")
                    nc.vector.tensor_tensor(
                        out=t1[:, 0:n, :], in0=r3[:, :, 0:256], in1=r3[:, :, 256:512],
                        op=ALU.add,
                    )
                    t2 = rp.tile([128, 4, 128], bf16, tag="t2")
                    nc.vector.tensor_tensor(
                        out=t2[:, 0:n, :], in0=t1[:, 0:n, 0:128], in1=t1[:, 0:n, 128:256],
                        op=ALU.add,
                    )
                    t3 = rp.tile([128, 4, 64], bf16, tag="t3")
                    nc.vector.tensor_tensor(
                        out=t3[:, 0:n, :], in0=t2[:, 0:n, 0:64], in1=t2[:, 0:n, 64:128],
                        op=ALU.add,
                    )
                    with nc.allow_low_precision("f32r feature sums"):
                        nc.vector.tensor_reduce(
                            out=s_all[:, offs : offs + n], in_=t3[:, 0:n, :],
                            axis=AX.X, op=ALU.add,
                        )
                    offs += n

            # dummy sqrt: switches the act table under the collective window
            # (reads the last exp output so it can't be hoisted before the exps)
            dsq = sp.tile([128, 1], f32)
            nc.scalar.activation(dsq[:], last_resp[:, 0:1], AF.Sqrt, bias=zero_t[:], scale=1.0)

            wr = mpack[:, WR_C : WR_C + HID]
            g1 = cpack[:, G1_C : G1_C + 1]
            b1 = cpack[:, B1_C : B1_C + 1]
            g2 = cpack[:, G2_C : G2_C + 1]
            b2 = cpack[:, B2_C : B2_C + 1]
            w2 = mpack[:, W2_C : W2_C + OUT]
            ones128 = mpack[:, ONES_C : ONES_C + 1]
            ones1 = mpack[0:1, ONES1_C : ONES1_C + 128]

            # ================= head phase =================
            with tc.tile_pool(name="pst", bufs=1, space="PSUM") as pt:
                # gather the raw chunk sums; the rmat chunk-combine is fused
                # into WR = rmat @ W1 host-side
                xb = dp.tile([128, BL], f32r, name="xb")
                nc.sync.dma_start(out=xb[:], in_=s_all[:])
                xg = dp.tile([N_CORES * 128, BL], f32r, name="xg", addr_space="Shared")
                cc = nc.gpsimd.collective_compute(
                    "AllGather",
                    ALU.bypass,
                    replica_groups=groups,
                    ins=[xb[:]],
                    outs=[xg[:]],
                )
                # express the same buffers as flat 1-D APs (lowered without
                # opt) so dim 0 carries the full extent
                cc.ins.ins[0] = nc.gpsimd.lower_ap(
                    xb[:].rearrange("r (c o) -> (r c) o", o=1), opt=False
                )
                cc.ins.outs[0] = nc.gpsimd.lower_ap(
                    xg[:].rearrange("r (c o) -> (r c) o", o=1), opt=False
                )

                # keep the PE p-state hot through the collective; the tiny
                # copy makes the dummies depend on the last-written s_all
                # column so they cannot preempt the head in the PE wait queue
                nc.vector.tensor_copy(zero_s[0:1, 0:1], s_all[0:1, BL - 1 : BL])
                dmt = pt.tile([128, 512], f32, tag="dummy")
                for _ in range(NDUMMY):
                    nc.tensor.matmul(dmt[:], zero_s[:, 0:128], zero_s[:])

                # gather -> xtf [128 chunkfeat, 256 batch] (batch = core-major)
                xtf = sp.tile([128, N_CORES, BL], f32r)
                nc.gpsimd.dma_start(
                    out=xtf[:], in_=xg[:].rearrange("(c f) b -> f c b", c=N_CORES)
                )
                xtf2 = xtf[:].rearrange("f c b -> f (c b)")

                # u = (rmat @ W1)^T s -> [128 hid, 256 batch]
                u_ps = pt.tile([HID, B], f32, tag="u")
                nc.tensor.matmul(u_ps[:], wr, xtf2)

                # BN1 stats (exact, global over the gathered batch)
                st1 = sp.tile([HID, 6], f32)
                nc.vector.bn_stats(st1[:], u_ps[:])
                mv1 = sp.tile([HID, 2], f32)
                nc.vector.bn_aggr(mv1[:], st1[:])
                sd1 = sp.tile([HID, 1], f32)
                nc.scalar.activation(sd1[:], mv1[:, 1:2], AF.Sqrt, bias=eps_t[:])
                rs1 = sp.tile([HID, 1], f32)
                nc.vector.reciprocal(rs1[:], sd1[:])
                a1 = sp.tile([HID, 1], f32)
                nc.vector.tensor_tensor(out=a1[:], in0=rs1[:], in1=g1, op=ALU.mult)
                nb1 = sp.tile([HID, 1], f32)
                nc.vector.tensor_tensor(out=nb1[:], in0=mv1[:, 0:1], in1=a1[:], op=ALU.mult)
                nc.vector.tensor_tensor(out=nb1[:], in0=b1, in1=nb1[:], op=ALU.subtract)

                # h = relu(a1 * u + nb1)
                h = rp.tile([HID, B], f32r, tag="h")
                nc.scalar.activation(h[:], u_ps[:], AF.Relu, bias=nb1[:], scale=a1[:])

                # y = h @ W2 -> [128 out, 256 batch]
                y_ps = pt.tile([OUT, B], f32, tag="y")
                nc.tensor.matmul(y_ps[:], w2, h[:])

                st2 = sp.tile([OUT, 6], f32)
                nc.vector.bn_stats(st2[:], y_ps[:])
                mv2 = sp.tile([OUT, 2], f32)
                nc.vector.bn_aggr(mv2[:], st2[:])
                sd2 = sp.tile([OUT, 1], f32)
                nc.scalar.activation(sd2[:], mv2[:, 1:2], AF.Sqrt, bias=eps_t[:])
                rs2 = sp.tile([OUT, 1], f32)
                nc.vector.reciprocal(rs2[:], sd2[:])
                a2 = sp.tile([OUT, 1], f32)
                nc.vector.tensor_tensor(out=a2[:], in0=rs2[:], in1=g2, op=ALU.mult)
                nb2 = sp.tile([OUT, 1], f32)
                nc.vector.tensor_tensor(out=nb2[:], in0=mv2[:, 0:1], in1=a2[:], op=ALU.mult)
                nc.vector.tensor_tensor(out=nb2[:], in0=b2, in1=nb2[:], op=ALU.subtract)

                # y_bn = a2*y + nb2 ; L2-normalize columns. ysq is computed
                # first, fused from y_ps, so the q/sdq chain overlaps y_bn.
                ysq = rp.tile([OUT, B], f32r, tag="ysq")
                nc.scalar.activation(ysq[:], y_ps[:], AF.Square, bias=nb2[:], scale=a2[:])
                q_ps = pt.tile([1, B], f32, tag="q")
                nc.tensor.matmul(q_ps[:], ones128, ysq[:])
                sdq = sp.tile([1, B], f32)
                nc.scalar.activation(sdq[:], q_ps[:], AF.Sqrt, bias=zero_t[0:1, :])
                rq = sp.tile([1, B], f32r)
                with nc.allow_low_precision("f32r norm scale"):
                    nc.vector.reciprocal(rq[:], sdq[:])
                y_bn = rp.tile([OUT, B], f32, tag="ybn")
                nc.scalar.activation(y_bn[:], y_ps[:], AF.Identity, bias=nb2[:], scale=a2[:])
                rnb_ps = pt.tile([128, B], f32, tag="rnb")
                nc.tensor.matmul(rnb_ps[:], ones1, rq[:])
                out_sb = rp.tile([OUT, B], f32, tag="osb")
                nc.vector.tensor_tensor(
                    out=out_sb[:], in0=y_bn[:], in1=rnb_ps[:], op=ALU.mult
                )
                nc.sync.dma_start(out=out_d[:], in_=out_sb[:])

    nc.finalize()
    return nc


def _get_nc():
    if "nc" not in _CACHE:
        _CACHE["nc"] = _build()
    return _CACHE["nc"]


def _softplus(x):
    return np.logaddexp(0.0, x)


def _prep_weights(centers0, log_sharp0, centers1, log_sharp1, W1, W2,
                  gamma1, beta1, gamma2, beta2):
    """Host-side packing of the tiny SLayer/head params."""
    import ml_dtypes

    s0 = _softplus(log_sharp0.astype(np.float64)) + 1e-6  # [E,2]
    s1 = _softplus(log_sharp1.astype(np.float64)) + 1e-6
    c0 = centers0.astype(np.float64)
    c1 = centers1.astype(np.float64)

    # 40 rhs rows per batch: j=0..7 x of group j (0..3 h0 chunks, 4..7 h1),
    # 8..15 y, 16..23 x^2, 24..31 y^2, 32..39 mask. outputs 16G+e.
    w1b = np.zeros((40, 128), np.float64)
    ebias = np.zeros(128, np.float64)
    for G in range(8):
        me = 16 * G + np.arange(E)
        sx, sy = (s0[:, 0], s0[:, 1]) if G < 4 else (s1[:, 0], s1[:, 1])
        cx, cy = (c0[:, 0], c0[:, 1]) if G < 4 else (c1[:, 0], c1[:, 1])
        w1b[G, me] = 2.0 * sx * cx
        w1b[8 + G, me] = 2.0 * sy * cy
        w1b[16 + G, me] = -sx
        w1b[24 + G, me] = -sy
        w1b[32 + G, me] = MASK_W
        ebias[me] = -(sx * cx**2 + sy * cy**2)

    # weight hi/lo residual: the kernel runs two accumulating matmuls
    # (w_hi then w_lo) against the same bf16 data rows
    w_hi = w1b.astype(ml_dtypes.bfloat16).astype(np.float64)
    w_lo = w1b - w_hi
    w40 = np.zeros((40, 256), np.float64)
    w40[:, 0:128] = w_hi
    w40[:, 128:256] = w_lo

    rmat = np.zeros((128, 32), np.float32)
    for g in range(4):
        for e in range(E):
            rmat[16 * g + e, e] = 1.0
            rmat[64 + 16 * g + e, 16 + e] = 1.0

    cpack = np.zeros((128, CPACK_W), np.float32)
    cpack[:, EB_C] = ebias.astype(np.float32)
    cpack[:, G1_C] = np.asarray(gamma1, np.float32)
    cpack[:, B1_C] = np.asarray(beta1, np.float32)
    cpack[:, G2_C] = np.asarray(gamma2, np.float32)
    cpack[:, B2_C] = np.asarray(beta2, np.float32)

    mpack = np.zeros((128, MPACK_W), np.float32)
    mpack[:, WR_C : WR_C + HID] = (
        rmat.astype(np.float64) @ np.asarray(W1, np.float64)
    ).astype(np.float32)
    mpack[:, W2_C : W2_C + OUT] = np.asarray(W2, np.float32)
    mpack[:, ONES_C] = 1.0
    mpack[0, ONES1_C : ONES1_C + 128] = 1.0

    return w40.astype(ml_dtypes.bfloat16), cpack, mpack


def _pack_core(bc0, bc1, c0, c1):
    """Pack one core's barcodes into a [128, NSUP*CH] bf16 rhs supertile row."""
    import ml_dtypes

    BLc = bc0.shape[0]
    blk = np.zeros((BLc, 128, CH), np.float32)  # per-batch rows at base 0
    pidx = np.arange(P, dtype=np.float32).reshape(NCH, CH)  # global point index
    for h, (bc, cnt) in enumerate(((bc0, c0), (bc1, c1))):
        xy = bc.reshape(BLc, NCH, CH, 2)
        g0 = 4 * h
        blk[:, g0 : g0 + 4] = xy[..., 0]
        blk[:, 8 + g0 : 12 + g0] = xy[..., 1]
        blk[:, 16 + g0 : 20 + g0] = xy[..., 0] ** 2
        blk[:, 24 + g0 : 28 + g0] = xy[..., 1] ** 2
        mask = (pidx[None] >= cnt[:, None, None]).astype(np.float32)
        blk[:, 32 + g0 : 36 + g0] = mask
    # one batch per supertile, 40 bf16 data rows
    sup = blk[:, 0:40].astype(ml_dtypes.bfloat16)
    # supertile-major along the free dim: [40, NSUP*CH]
    return np.ascontiguousarray(sup.transpose(1, 0, 2).reshape(40, BLc * CH))


def kernel(
    barcode_h0,
    barcode_h0_count,
    barcode_h1,
    barcode_h1_count,
    centers0,
    log_sharp0,
    centers1,
    log_sharp1,
    W1,
    gamma1,
    beta1,
    W2,
    gamma2,
    beta2,
):
    from concourse.bass_utils import run_bass_kernel_spmd

    nc = _get_nc()
    w40, cpack, mpack = _prep_weights(
        centers0, log_sharp0, centers1, log_sharp1, W1, W2,
        gamma1, beta1, gamma2, beta2,
    )

    bc0 = np.ascontiguousarray(barcode_h0, dtype=np.float32)
    bc1 = np.ascontiguousarray(barcode_h1, dtype=np.float32)
    c0 = np.asarray(barcode_h0_count).astype(np.float32)
    c1 = np.asarray(barcode_h1_count).astype(np.float32)

    in_maps = []
    for c in range(N_CORES):
        sl = slice(c * BL, (c + 1) * BL)
        in_maps.append(
            {
                "xpack": _pack_core(bc0[sl], bc1[sl], c0[sl], c1[sl]),
                "wpack": w40,
                "cpack": cpack,
                "mpack": mpack,
            }
        )

    _CACHE["in_maps"] = in_maps
    res = run_bass_kernel_spmd(nc, in_maps, core_ids=list(range(N_CORES)))
    out = res.results[0]["out"]  # [OUT, B]
    return np.ascontiguousarray(out.T).astype(np.float32)


# revision 24
# speedup vs baseline: 1.0324x; 1.0138x over previous
"""Trainium2 Bass kernel for nn_BarcodeSLayerEncoder.

Design (8 NeuronCores, pure data-parallel over batch):
  - Each core gets B/8 = 32 batches of both barcode tensors.
  - SLayer logits via TensorE matmuls (K=120, 512 cols per batch):
      logit[16g+e, p] = 2*s_x*c_x[e]*x - s_x*x^2 + 2*s_y*c_y[e]*y - s_y*y^2
                        - 100*maskflag(g,p)          (g = 4 h0-chunks + 4 h1-chunks)
    with the per-center constant -s*||c||^2 folded into the ScalarE Exp bias.
    Supertiles are processed in blocks of up to 4 batches so one wide Exp
    activation covers [128, 2048] (amortizes the ACT access latency).
  - Point sums: 3 bf16 TensorTensor halvings on DVE (4x perf mode) then one
    grouped f32 tensor_reduce -> s_all f32 (bf16 rounding stays ~1e-3 overall).
  - Features x2 [32,32] f32 -> AllGather -> every core redundantly computes
    the tiny head with exact global BN stats (bn_stats/bn_aggr for mean+var).
    Head matmuls use float32r (full precision, 1 cycle/row at >=256 cols).
  - Act tables: a dummy Exp at t=0 preloads the exp table under the DMA
    prologue; a dummy Sqrt right after the last Exp switches to the sqrt
    table under the collective. Dummy matmuls keep the PE p-state at full
    clock through the collective so the head matmuls run at 2.4 GHz.
"""

import sys

sys.path.insert(0, "/opt/trn_rl_repo")

import numpy as np

N_CORES = 8
B, P, E, D = 256, 2048, 16, 2
BL = B // N_CORES  # 32 local batches per core
HID, OUT = 128, 128
BN_EPS = 1e-5
CH = 512  # point chunk size (one PSUM bank of f32)
NCH = P // CH  # 4 chunks per homology
NSUP = BL  # one batch per supertile
MASK_W = -100.0
BLOCK_SIZES = [1, 2, 3, 4, 4, 4, 4, 4, 3, 1, 1, 1]  # supertiles per exp block
NDUMMY = 86  # PE keep-warm matmuls spanning the collective window

# cpack (f32) column layout: ebias, g1, b1, g2, b2
EB_C = 0
G1_C, B1_C, G2_C, B2_C = 1, 2, 3, 4
CPACK_W = 5
# mpack (f32r) column layout: matmul weights for the head
WR_C = 0  # [0:128] WR = rmat @ W1 (chunk-combine fused into the u matmul)
W2_C = 128  # [128:256] W2
ONES_C = 256  # ones column
ONES1_C = 257  # row 0, [257:385] ones row
MPACK_W = 385

_CACHE = {}


def _build():
    from concourse import bacc, bass, mybir, tile

    f32 = mybir.dt.float32
    f32r = mybir.dt.float32r
    bf16 = mybir.dt.bfloat16
    nc = bacc.Bacc("TRN2", target_bir_lowering=False, debug=False)

    # ---- kernel I/O ----
    xin = nc.declare_dram_parameter("xpack", [40, NSUP * CH], bf16, isOutput=False)
    wpack_d = nc.declare_dram_parameter("wpack", [40, 256], bf16, isOutput=False)
    cpack_d = nc.declare_dram_parameter("cpack", [128, CPACK_W], f32, isOutput=False)
    mpack_d = nc.declare_dram_parameter("mpack", [128, MPACK_W], f32r, isOutput=False)
    out_d = nc.declare_dram_parameter("out", [OUT, B], f32, isOutput=True)

    AF = mybir.ActivationFunctionType
    ALU = mybir.AluOpType
    AX = mybir.AxisListType
    groups = [list(range(N_CORES))]

    with tile.TileContext(nc) as tc:
        with (
            tc.tile_pool(name="consts", bufs=1) as cp,
            tc.tile_pool(name="xyt", bufs=3) as xyp,
            tc.tile_pool(name="resp", bufs=2) as rp,
            tc.tile_pool(name="small", bufs=1) as sp,
            tc.tile_pool(name="dram", bufs=1, space="DRAM") as dp,
        ):
            zero_t = cp.tile([128, 1], f32)
            nc.gpsimd.memset(zero_t[:], 0.0)
            eps_t = cp.tile([128, 1], f32)
            nc.gpsimd.memset(eps_t[:], BN_EPS)
            zero_s = cp.tile([128, 512], bf16)
            nc.gpsimd.memset(zero_s[:], 0.0)

            # dummy exp: preloads the exp act table under the DMA prologue
            d0 = sp.tile([128, 1], bf16)
            nc.scalar.activation(d0[:], zero_t[:], AF.Exp, bias=zero_t[:], scale=1.0)

            # ---- batched constant loads ----
            wpack = cp.tile([40, 256], bf16)
            nc.sync.dma_start(out=wpack[:], in_=wpack_d[:])
            cpack = cp.tile([128, CPACK_W], f32)
            nc.sync.dma_start(out=cpack[:], in_=cpack_d[:])
            ebias_t = cpack[:, EB_C : EB_C + 1]
            w40hi = wpack[0:40, 0:128]
            w40lo = wpack[0:40, 128:256]

            s_all = sp.tile([128, BL], f32r)  # per-batch chunk-partial SLayer sums
            mpack = cp.tile([128, MPACK_W], f32r)

            # ================= SLayer phase =================
            with tc.tile_pool(name="pslayer", bufs=2, space="PSUM") as pp:
                offs = 0
                for bi, n in enumerate(BLOCK_SIZES):
                    w = n * CH
                    xyt = xyp.tile([128, 4 * CH], bf16, tag="xyt", name=f"xyt{bi}")
                    eng = (nc.sync, nc.gpsimd)[bi % 2]
                    eng.dma_start(
                        out=xyt[0:40, 0:w], in_=xin[:, offs * CH : offs * CH + w]
                    )
                    if bi == 3:
                        # mpack is only needed from the u matmul on; load it
                        # well after the pipeline ramp
                        nc.sync.dma_start(out=mpack[:], in_=mpack_d[:])
                    ps = pp.tile([128, 4 * CH], f32, tag="lg")
                    for j in range(n):
                        rhs = xyt[0:40, j * CH : (j + 1) * CH]
                        if bi == 0:
                            nc.tensor.matmul(
                                ps[:, j * CH : (j + 1) * CH], w40hi, rhs,
                                start=True, stop=True,
                            )
                            continue
                        nc.tensor.matmul(
                            ps[:, j * CH : (j + 1) * CH], w40hi, rhs,
                            start=True, stop=False,
                        )
                        nc.tensor.matmul(
                            ps[:, j * CH : (j + 1) * CH], w40lo, rhs,
                            start=False, stop=True,
                        )
                    resp = rp.tile([128, 4 * CH], bf16, tag="resp")
                    last_resp = resp
                    if n == 1 and bi >= 9:
                        # tail blocks: point-sum straight off the Exp's
                        # accumulator so the DVE tree isn't on the s_all path
                        with nc.allow_low_precision("f32r feature sums"):
                            nc.scalar.activation(
                                resp[:, 0:w], ps[:, 0:w], AF.Exp, bias=ebias_t,
                                scale=1.0, accum_out=s_all[:, offs : offs + 1],
                            )
                        offs += n
                        continue
                    nc.scalar.activation(
                        resp[:, 0:w], ps[:, 0:w], AF.Exp, bias=ebias_t, scale=1.0
                    )
                    # bf16 halving tree on DVE (4x mode), then grouped f32 reduce
                    r3 = resp[:, 0:w].rearrange("p (s c) -> p s c", s=n)
                    t1 = rp.tile([128, 4, 256], bf16, tag="t1")
                    nc.vector.tensor_tensor(
                        out=t1[:, 0:n, :], in0=r3[:, :, 0:256], in1=r3[:, :, 256:512],
                        op=ALU.add,
                    )
                    t2 = rp.tile([128, 4, 128], bf16, tag="t2")
                    nc.vector.tensor_tensor(
                        out=t2[:, 0:n, :], in0=t1[:, 0:n, 0:128], in1=t1[:, 0:n, 128:256],
                        op=ALU.add,
                    )
                    t3 = rp.tile([128, 4, 64], bf16, tag="t3")
                    nc.vector.tensor_tensor(
                        out=t3[:, 0:n, :], in0=t2[:, 0:n, 0:64], in1=t2[:, 0:n, 64:128],
                        op=ALU.add,
                    )
                    with nc.allow_low_precision("f32r feature sums"):
                        nc.vector.tensor_reduce(
                            out=s_all[:, offs : offs + n], in_=t3[:, 0:n, :],
                            axis=AX.X, op=ALU.add,
                        )
                    offs += n

            # dummy sqrt: switches the act table under the collective window
            # (reads the last exp output so it can't be hoisted before the exps)
            dsq = sp.tile([128, 1], f32)
            nc.scalar.activation(dsq[:], last_resp[:, 0:1], AF.Sqrt, bias=zero_t[:], scale=1.0)

            wr = mpack[:, WR_C : WR_C + HID]
            g1 = cpack[:, G1_C : G1_C + 1]
            b1 = cpack[:, B1_C : B1_C + 1]
            g2 = cpack[:, G2_C : G2_C + 1]
            b2 = cpack[:, B2_C : B2_C + 1]
            w2 = mpack[:, W2_C : W2_C + OUT]
            ones128 = mpack[:, ONES_C : ONES_C + 1]
            ones1 = mpack[0:1, ONES1_C : ONES1_C + 128]

            # ================= head phase =================
            with tc.tile_pool(name="pst", bufs=1, space="PSUM") as pt:
                # gather the raw chunk sums; the rmat chunk-combine is fused
                # into WR = rmat @ W1 host-side
                xb = dp.tile([128, BL], f32r, name="xb")
                nc.sync.dma_start(out=xb[:], in_=s_all[:])
                xg = dp.tile([N_CORES * 128, BL], f32r, name="xg", addr_space="Shared")
                cc = nc.gpsimd.collective_compute(
                    "AllGather",
                    ALU.bypass,
                    replica_groups=groups,
                    ins=[xb[:]],
                    outs=[xg[:]],
                )
                # express the same buffers as flat 1-D APs (lowered without
                # opt) so dim 0 carries the full extent
                cc.ins.ins[0] = nc.gpsimd.lower_ap(
                    xb[:].rearrange("r (c o) -> (r c) o", o=1), opt=False
                )
                cc.ins.outs[0] = nc.gpsimd.lower_ap(
                    xg[:].rearrange("r (c o) -> (r c) o", o=1), opt=False
                )

                # keep the PE p-state hot through the collective; the tiny
                # copy makes the dummies depend on the last-written s_all
                # column so they cannot preempt the head in the PE wait queue
                nc.vector.tensor_copy(zero_s[0:1, 0:1], s_all[0:1, BL - 1 : BL])
                dmt = pt.tile([128, 512], f32, tag="dummy")
                for _ in range(NDUMMY):
                    nc.tensor.matmul(dmt[:], zero_s[:, 0:128], zero_s[:])

                # gather -> xtf [128 chunkfeat, 256 batch] (batch = core-major)
                xtf = sp.tile([128, N_CORES, BL], f32r)
                nc.gpsimd.dma_start(
                    out=xtf[:], in_=xg[:].rearrange("(c f) b -> f c b", c=N_CORES)
                )
                xtf2 = xtf[:].rearrange("f c b -> f (c b)")

                # u = (rmat @ W1)^T s -> [128 hid, 256 batch]
                u_ps = pt.tile([HID, B], f32, tag="u")
                nc.tensor.matmul(u_ps[:], wr, xtf2)

                # BN1 stats (exact, global over the gathered batch)
                st1 = sp.tile([HID, 6], f32)
                nc.vector.bn_stats(st1[:], u_ps[:])
                mv1 = sp.tile([HID, 2], f32)
                nc.vector.bn_aggr(mv1[:], st1[:])
                sd1 = sp.tile([HID, 1], f32)
                nc.scalar.activation(sd1[:], mv1[:, 1:2], AF.Sqrt, bias=eps_t[:])
                rs1 = sp.tile([HID, 1], f32)
                nc.vector.reciprocal(rs1[:], sd1[:])
                a1 = sp.tile([HID, 1], f32)
                nc.vector.tensor_tensor(out=a1[:], in0=rs1[:], in1=g1, op=ALU.mult)
                nb1 = sp.tile([HID, 1], f32)
                nc.vector.tensor_tensor(out=nb1[:], in0=mv1[:, 0:1], in1=a1[:], op=ALU.mult)
                nc.vector.tensor_tensor(out=nb1[:], in0=b1, in1=nb1[:], op=ALU.subtract)

                # h = relu(a1 * u + nb1)
                h = rp.tile([HID, B], f32r, tag="h")
                nc.scalar.activation(h[:], u_ps[:], AF.Relu, bias=nb1[:], scale=a1[:])

                # y = h @ W2 -> [128 out, 256 batch]
                y_ps = pt.tile([OUT, B], f32, tag="y")
                nc.tensor.matmul(y_ps[:], w2, h[:])

                st2 = sp.tile([OUT, 6], f32)
                nc.vector.bn_stats(st2[:], y_ps[:])
                mv2 = sp.tile([OUT, 2], f32)
                nc.vector.bn_aggr(mv2[:], st2[:])
                sd2 = sp.tile([OUT, 1], f32)
                nc.scalar.activation(sd2[:], mv2[:, 1:2], AF.Sqrt, bias=eps_t[:])
                rs2 = sp.tile([OUT, 1], f32)
                nc.vector.reciprocal(rs2[:], sd2[:])
                a2 = sp.tile([OUT, 1], f32)
                nc.vector.tensor_tensor(out=a2[:], in0=rs2[:], in1=g2, op=ALU.mult)
                nb2 = sp.tile([OUT, 1], f32)
                nc.vector.tensor_tensor(out=nb2[:], in0=mv2[:, 0:1], in1=a2[:], op=ALU.mult)
                nc.vector.tensor_tensor(out=nb2[:], in0=b2, in1=nb2[:], op=ALU.subtract)

                # y_bn = a2*y + nb2 ; L2-normalize columns. ysq is computed
                # first, fused from y_ps, so the q/sdq chain overlaps y_bn.
                ysq = rp.tile([OUT, B], f32r, tag="ysq")
                nc.scalar.activation(ysq[:], y_ps[:], AF.Square, bias=nb2[:], scale=a2[:])
                q_ps = pt.tile([1, B], f32, tag="q")
                nc.tensor.matmul(q_ps[:], ones128, ysq[:])
                sdq = sp.tile([1, B], f32)
                nc.scalar.activation(sdq[:], q_ps[:], AF.Sqrt, bias=zero_t[0:1, :])
                rq = sp.tile([1, B], f32r)
                with nc.allow_low_precision("f32r norm scale"):
                    nc.vector.reciprocal(rq[:], sdq[:])
                y_bn = rp.tile([OUT, B], f32, tag="ybn")
                nc.scalar.activation(y_bn[:], y_ps[:], AF.Identity, bias=nb2[:], scale=a2[:])
                rnb_ps = pt.tile([128, B], f32, tag="rnb")
                nc.tensor.matmul(rnb_ps[:], ones1, rq[:])
                out_sb = rp.tile([OUT, B], f32, tag="osb")
                nc.vector.tensor_tensor(
                    out=out_sb[:], in0=y_bn[:], in1=rnb_ps[:], op=ALU.mult
                )
                nc.sync.dma_start(out=out_d[:], in_=out_sb[:])

    nc.finalize()
    return nc


def _get_nc():
    if "nc" not in _CACHE:
        _CACHE["nc"] = _build()
    return _CACHE["nc"]


def _softplus(x):
    return np.logaddexp(0.0, x)


def _prep_weights(centers0, log_sharp0, centers1, log_sharp1, W1, W2,
                  gamma1, beta1, gamma2, beta2):
    """Host-side packing of the tiny SLayer/head params."""
    import ml_dtypes

    s0 = _softplus(log_sharp0.astype(np.float64)) + 1e-6  # [E,2]
    s1 = _softplus(log_sharp1.astype(np.float64)) + 1e-6
    c0 = centers0.astype(np.float64)
    c1 = centers1.astype(np.float64)

    # 40 rhs rows per batch: j=0..7 x of group j (0..3 h0 chunks, 4..7 h1),
    # 8..15 y, 16..23 x^2, 24..31 y^2, 32..39 mask. outputs 16G+e.
    w1b = np.zeros((40, 128), np.float64)
    ebias = np.zeros(128, np.float64)
    for G in range(8):
        me = 16 * G + np.arange(E)
        sx, sy = (s0[:, 0], s0[:, 1]) if G < 4 else (s1[:, 0], s1[:, 1])
        cx, cy = (c0[:, 0], c0[:, 1]) if G < 4 else (c1[:, 0], c1[:, 1])
        w1b[G, me] = 2.0 * sx * cx
        w1b[8 + G, me] = 2.0 * sy * cy
        w1b[16 + G, me] = -sx
        w1b[24 + G, me] = -sy
        w1b[32 + G, me] = MASK_W
        ebias[me] = -(sx * cx**2 + sy * cy**2)

    # weight hi/lo residual: the kernel runs two accumulating matmuls
    # (w_hi then w_lo) against the same bf16 data rows
    w_hi = w1b.astype(ml_dtypes.bfloat16).astype(np.float64)
    w_lo = w1b - w_hi
    w40 = np.zeros((40, 256), np.float64)
    w40[:, 0:128] = w_hi
    w40[:, 128:256] = w_lo

    rmat = np.zeros((128, 32), np.float32)
    for g in range(4):
        for e in range(E):
            rmat[16 * g + e, e] = 1.0
            rmat[64 + 16 * g + e, 16 + e] = 1.0

    cpack = np.zeros((128, CPACK_W), np.float32)
    cpack[:, EB_C] = ebias.astype(np.float32)
    cpack[:, G1_C] = np.asarray(gamma1, np.float32)
    cpack[:, B1_C] = np.asarray(beta1, np.float32)
    cpack[:, G2_C] = np.asarray(gamma2, np.float32)
    cpack[:, B2_C] = np.asarray(beta2, np.float32)

    mpack = np.zeros((128, MPACK_W), np.float32)
    mpack[:, WR_C : WR_C + HID] = (
        rmat.astype(np.float64) @ np.asarray(W1, np.float64)
    ).astype(np.float32)
    mpack[:, W2_C : W2_C + OUT] = np.asarray(W2, np.float32)
    mpack[:, ONES_C] = 1.0
    mpack[0, ONES1_C : ONES1_C + 128] = 1.0

    return w40.astype(ml_dtypes.bfloat16), cpack, mpack


def _pack_core(bc0, bc1, c0, c1):
    """Pack one core's barcodes into a [128, NSUP*CH] bf16 rhs supertile row."""
    import ml_dtypes

    BLc = bc0.shape[0]
    blk = np.zeros((BLc, 128, CH), np.float32)  # per-batch rows at base 0
    pidx = np.arange(P, dtype=np.float32).reshape(NCH, CH)  # global point index
    for h, (bc, cnt) in enumerate(((bc0, c0), (bc1, c1))):
        xy = bc.reshape(BLc, NCH, CH, 2)
        g0 = 4 * h
        blk[:, g0 : g0 + 4] = xy[..., 0]
        blk[:, 8 + g0 : 12 + g0] = xy[..., 1]
        blk[:, 16 + g0 : 20 + g0] = xy[..., 0] ** 2
        blk[:, 24 + g0 : 28 + g0] = xy[..., 1] ** 2
        mask = (pidx[None] >= cnt[:, None, None]).astype(np.float32)
        blk[:, 32 + g0 : 36 + g0] = mask
    # one batch per supertile, 40 bf16 data rows
    sup = blk[:, 0:40].astype(ml_dtypes.bfloat16)
    # supertile-major along the free dim: [40, NSUP*CH]
    return np.ascontiguousarray(sup.transpose(1, 0, 2).reshape(40, BLc * CH))


def kernel(
    barcode_h0,
    barcode_h0_count,
    barcode_h1,
    barcode_h1_count,
    centers0,
    log_sharp0,
    centers1,
    log_sharp1,
    W1,
    gamma1,
    beta1,
    W2,
    gamma2,
    beta2,
):
    from concourse.bass_utils import run_bass_kernel_spmd

    nc = _get_nc()
    w40, cpack, mpack = _prep_weights(
        centers0, log_sharp0, centers1, log_sharp1, W1, W2,
        gamma1, beta1, gamma2, beta2,
    )

    bc0 = np.ascontiguousarray(barcode_h0, dtype=np.float32)
    bc1 = np.ascontiguousarray(barcode_h1, dtype=np.float32)
    c0 = np.asarray(barcode_h0_count).astype(np.float32)
    c1 = np.asarray(barcode_h1_count).astype(np.float32)

    in_maps = []
    for c in range(N_CORES):
        sl = slice(c * BL, (c + 1) * BL)
        in_maps.append(
            {
                "xpack": _pack_core(bc0[sl], bc1[sl], c0[sl], c1[sl]),
                "wpack": w40,
                "cpack": cpack,
                "mpack": mpack,
            }
        )

    _CACHE["in_maps"] = in_maps
    res = run_bass_kernel_spmd(nc, in_maps, core_ids=list(range(N_CORES)))
    out = res.results[0]["out"]  # [OUT, B]
    return np.ascontiguousarray(out.T).astype(np.float32)
